# revision 2
# baseline (speedup 1.0000x reference)
"""Trainium2 Bass kernel for causal cross-attention with L2-normalized q/k.

Reference computation (B=4, S=2048, E=512, H=8, Dh=64):
    q = k_embed @ Wq.T ; k = x @ Wk.T ; v = x @ Wv.T        (per batch)
    q,k l2-normalized over Dh per head; scores = g * q @ k.T (causal mask)
    out = softmax(scores) @ v

Sharding: 8 cores = 4 batches x 2 head-groups (4 heads each). Each core:
  - projections from host-pre-transposed xT / k_embedT (E on partition dim);
    v and k share one fused matmul (host-concatenated weights)
  - q,k l2-normalized in natural layout (per-partition ops) then
    PE-transposed into [head_dim, seq] layout
  - scores computed transposed: st[sk, sq]; the softmax denominator comes
    from a ones-row appended to v; heads processed in pairs so the two
    K=64 score matmuls occupy disjoint PE row groups (concurrent)
  - returns outT [4 heads, 65, S] (64 rows of unnormalized out^T + the
    softmax denominator row); host divides and reassembles.
"""

import numpy as np

B, S, E, H = 4, 2048, 512, 8
Dh = 64
NE = E // 128          # 4 contraction chunks
NT = S // 128          # 16 s-tiles
SQB = 1024             # sq block width (2 PSUM banks)
NJ = S // SQB          # 2


def _build(g: float, repeats: int = 1):
    from contextlib import ExitStack

    import concourse.tile as tile
    from concourse import bacc, mybir
    from concourse.masks import make_identity

    f32 = mybir.dt.float32
    AF = mybir.ActivationFunctionType
    ALU = mybir.AluOpType

    nc = bacc.Bacc("TRN2", target_bir_lowering=False, debug=False)
    xT_d = nc.dram_tensor("xt", [E, S], f32, kind="ExternalInput")
    keT_d = nc.dram_tensor("ket", [E, S], f32, kind="ExternalInput")
    wqT_d = nc.dram_tensor("wqt", [E, 256], f32, kind="ExternalInput")
    wvkT_d = nc.dram_tensor("wvkt", [E, 512], f32, kind="ExternalInput")
    mask_d = nc.dram_tensor("mask01", [128, 128], f32, kind="ExternalInput")
    out_d = nc.dram_tensor("outt", [4, 65, S], f32, kind="ExternalOutput")

    with tile.TileContext(nc) as tc:
     for _rep in range(repeats):
      with ExitStack() as ctx:
        persist = ctx.enter_context(tc.tile_pool(name=f"persist{_rep}", bufs=1))
        ptmp = ctx.enter_context(tc.tile_pool(name=f"ptmp{_rep}", bufs=3))
        ep = ctx.enter_context(tc.tile_pool(name=f"e_pool{_rep}", bufs=4))
        osb = ctx.enter_context(tc.tile_pool(name=f"o_sb{_rep}", bufs=2))

        # ---- input DMAs ----
        wq_sb = persist.tile([128, NE, 256], f32, tag="wq")
        wvk_sb = persist.tile([128, NE, 512], f32, tag="wvk")
        nc.sync.dma_start(out=wvk_sb, in_=wvkT_d.rearrange("(c p) m -> p c m", p=128))
        nc.sync.dma_start(out=wq_sb, in_=wqT_d.rearrange("(c p) m -> p c m", p=128))
        mask_sb = persist.tile([128, 128], f32, tag="mask")
        nc.sync.dma_start(out=mask_sb, in_=mask_d[:, :])

        x_sbs, ke_sbs = [], []
        for ec in range(NE):
            t = persist.tile([128, S], f32, tag=f"x{ec}", name=f"x{ec}")
            nc.sync.dma_start(out=t, in_=xT_d[ec * 128:(ec + 1) * 128, :])
            x_sbs.append(t)
        for ec in range(NE):
            t = persist.tile([128, S], f32, tag=f"ke{ec}", name=f"ke{ec}")
            nc.sync.dma_start(out=t, in_=keT_d[ec * 128:(ec + 1) * 128, :])
            ke_sbs.append(t)

        ident = persist.tile([128, 128], f32, tag="ident")
        make_identity(nc, ident[:, :])

        qt_sb = persist.tile([128, 2, S], f32, tag="qt")   # qhat^T per pair
        kt_sb = persist.tile([128, 2, S], f32, tag="kt")   # khat^T per pair
        v_sb = persist.tile([128, NT, 4, 65], f32, tag="v")  # v + ones col
        nc.vector.memset(v_sb[:, :, :, 64], 1.0)

        proj_ctx = ExitStack()
        pnp = proj_ctx.enter_context(
            tc.tile_pool(name=f"pn_ps{_rep}", bufs=3, space="PSUM"))
        ptp = proj_ctx.enter_context(
            tc.tile_pool(name=f"pt_ps{_rep}", bufs=4, space="PSUM"))

        def normalize_transpose(ps4, dst, st_i):
            """ps4: PSUM [128, 4, 64] natural proj; l2-normalize each 64-group
            and PE-transpose into dst[:, pair, st_i*128...]."""
            sq = ptmp.tile([128, 4, 64], f32, tag="sq", name="sq")
            nc.scalar.activation(sq[:, :, :], ps4[:, :, :], AF.Square)
            ss = ptmp.tile([128, 4], f32, tag="ss", name="ss")
            nc.vector.tensor_reduce(
                ss[:, :], sq[:, :, :], axis=mybir.AxisListType.X, op=ALU.add)
            rn = ptmp.tile([128, 4], f32, tag="rn", name="rn")
            nc.scalar.activation(rn[:, :], ss[:, :], AF.Sqrt)
            rq = ptmp.tile([128, 4], f32, tag="rq", name="rq")
            nc.vector.reciprocal(rq[:, :], rn[:, :])
            hat = ptmp.tile([128, 4, 64], f32, tag="hat", name="hat")
            for h4 in range(4):
                nc.vector.tensor_scalar_mul(
                    hat[:, h4, :], ps4[:, h4, :], rq[:, h4:h4 + 1])
            for pair in range(2):
                pst = ptp.tile([128, 128], f32, tag="t_ps", name="t_ps")
                nc.tensor.transpose(
                    pst[:, :], hat[:, 2 * pair:2 * pair + 2, :], ident[:, :])
                nc.vector.tensor_copy(
                    dst[:, pair, st_i * 128:(st_i + 1) * 128], pst[:, :])

        # ---- projections: v + k fused; q separate; k,q normalized then
        # transposed ----
        for st_i in range(NT):
            sl = slice(st_i * 128, (st_i + 1) * 128)
            psvk = pnp.tile([128, 512], f32, tag="nat", name="nat")
            for ec in range(NE):
                nc.tensor.matmul(
                    psvk[:, :], lhsT=x_sbs[ec][:, sl], rhs=wvk_sb[:, ec, :],
                    start=(ec == 0), stop=(ec == NE - 1))
            nc.vector.tensor_copy(
                v_sb[:, st_i, :, 0:64],
                psvk[:, 0:256].rearrange("p (h d) -> p h d", h=4))
            psk4 = psvk[:, 256:512].rearrange("p (h d) -> p h d", h=4)
            normalize_transpose(psk4, kt_sb, st_i)
        for st_i in range(NT):
            sl = slice(st_i * 128, (st_i + 1) * 128)
            psq = pnp.tile([128, 4, 64], f32, tag="nat", name="nat")
            for ec in range(NE):
                nc.tensor.matmul(
                    psq[:, :, :], lhsT=ke_sbs[ec][:, sl], rhs=wq_sb[:, ec, :],
                    start=(ec == 0), stop=(ec == NE - 1))
            normalize_transpose(psq, qt_sb, st_i)
        proj_ctx.close()

        # ---- attention: head pairs, transposed scores, ones-row denom ----
        with tc.tile_pool(name=f"st_ps{_rep}", bufs=2, space="PSUM") as stp, \
             tc.tile_pool(name=f"o_ps{_rep}", bufs=2, space="PSUM") as op:
            for pair in range(2):
                for j in range(NJ):
                    opsA = op.tile([65, SQB], f32, tag="o", name="o")
                    opsB = op.tile([65, SQB], f32, tag="o", name="o")
                    ops2 = (opsA, opsB)
                    for i in range(8 * j + 8):
                        co = max(0, 128 * i - SQB * j)
                        corner = 128 * i >= SQB * j
                        ksl = slice(i * 128, (i + 1) * 128)
                        sts2 = (stp.tile([128, SQB], f32, tag="st", name="st"),
                                stp.tile([128, SQB], f32, tag="st", name="st"))
                        # two heads of the pair: PE row groups 0-63 / 64-127
                        for c0, c1 in ((co, 512), (max(co, 512), SQB)):
                            if c0 >= c1:
                                continue
                            for hp in range(2):
                                psl = slice(hp * 64, (hp + 1) * 64)
                                nc.tensor.matmul(
                                    sts2[hp][:, c0:c1],
                                    lhsT=kt_sb[psl, pair, ksl],
                                    rhs=qt_sb[psl, pair, j * SQB + c0:j * SQB + c1],
                                    start=True, stop=True)
                        for hp in range(2):
                            hh = 2 * pair + hp
                            et = ep.tile([128, SQB], f32, tag="e", name="e")
                            nc.scalar.activation(
                                et[:, co:SQB], sts2[hp][:, co:SQB], AF.Exp,
                                scale=float(g))
                            if corner:
                                nc.vector.tensor_mul(
                                    et[:, co:co + 128], et[:, co:co + 128],
                                    mask_sb[:, :])
                            for a, b in ((co, 512), (max(co, 512), SQB)):
                                if a >= b:
                                    continue
                                # last matmul per PSUM bank: bank A final write
                                # is tile i == 8j+3 (corner), bank B's i == 8j+7
                                is_last = ((i == 8 * j + 3) if b <= 512
                                           else (i == 8 * j + 7))
                                nc.tensor.matmul(
                                    ops2[hp][:, a:b],
                                    lhsT=v_sb[:, i, hh, :],
                                    rhs=et[:, a:b],
                                    start=(i == 0), stop=is_last)
                    for hp in range(2):
                        hh = 2 * pair + hp
                        ot = osb.tile([65, SQB], f32, tag="osb", name="osb")
                        nc.vector.tensor_copy(ot[:, :], ops2[hp][:, :])
                        nc.sync.dma_start(
                            out_d[hh, :, j * SQB:(j + 1) * SQB], ot[:, :])
    nc.compile()
    return nc


_NC_CACHE = {}


def _get_nc(g: float):
    if g not in _NC_CACHE:
        _NC_CACHE[g] = _build(g)
    return _NC_CACHE[g]


def _numpy_fallback(x, k_embed, attn_mask, key_padding_mask, Wq, Wk, Wv, g_scale):
    def l2n(t):
        n = np.sqrt((t * t).sum(-1, keepdims=True))
        return t / np.maximum(n, 1e-12)
    q = (k_embed @ Wq.T).reshape(B, S, H, Dh).transpose(0, 2, 1, 3)
    k = (x @ Wk.T).reshape(B, S, H, Dh).transpose(0, 2, 1, 3)
    v = (x @ Wv.T).reshape(B, S, H, Dh).transpose(0, 2, 1, 3)
    q, k = l2n(q), l2n(k)
    s = float(g_scale) * np.einsum('bhqd,bhkd->bhqk', q, k)
    s = np.where(attn_mask[None, None], -np.inf, s)
    s = np.where(key_padding_mask[:, None, None, :], -np.inf, s)
    s = s - s.max(-1, keepdims=True)
    e = np.exp(s)
    a = e / e.sum(-1, keepdims=True)
    o = np.einsum('bhqk,bhkd->bhqd', a, v)
    return o.transpose(0, 2, 1, 3).reshape(B, S, E).astype(np.float32)


def _make_in_maps(x, k_embed, Wq, Wk, Wv):
    mask01 = np.triu(np.ones((128, 128), np.float32))  # keep sq >= sk
    in_maps = []
    for c in range(8):
        b, hg = c // 2, c % 2
        rows = slice(hg * 256, (hg + 1) * 256)
        wv_t = Wv[rows].T                       # [512, 256]
        wk_t = Wk[rows].T
        in_maps.append({
            "xt": np.ascontiguousarray(x[b].T),
            "ket": np.ascontiguousarray(k_embed[b].T),
            "wqt": np.ascontiguousarray(Wq[rows].T),
            "wvkt": np.ascontiguousarray(np.concatenate([wv_t, wk_t], axis=1)),
            "mask01": mask01,
        })
    return in_maps


def kernel(**inputs) -> np.ndarray:
    x = np.asarray(inputs["x"], np.float32)
    k_embed = np.asarray(inputs["k_embed"], np.float32)
    attn_mask = np.asarray(inputs["attn_mask"])
    key_padding_mask = np.asarray(inputs["key_padding_mask"])
    Wq = np.asarray(inputs["Wq"], np.float32)
    Wk = np.asarray(inputs["Wk"], np.float32)
    Wv = np.asarray(inputs["Wv"], np.float32)
    g = float(np.asarray(inputs["g_scale"]))

    causal = np.triu(np.ones((S, S), bool), k=1)
    if (attn_mask != causal).any() or key_padding_mask.any():
        return _numpy_fallback(x, k_embed, attn_mask, key_padding_mask,
                               Wq, Wk, Wv, g)

    import os

    from concourse.bass_utils import run_bass_kernel_spmd

    nc = _get_nc(g)
    in_maps = _make_in_maps(x, k_embed, Wq, Wk, Wv)
    res = run_bass_kernel_spmd(nc, in_maps, core_ids=list(range(8)),
                               tmpdir=os.environ.get("BASS_NEFF_DIR"))
    kernel._last_results = res

    out = np.empty((B, S, E), np.float32)
    for c in range(8):
        b, hg = c // 2, c % 2
        r = res.results[c]["outt"]          # [4, 65, S]
        w = r[:, :64, :] / r[:, 64:65, :]   # normalize by softmax denom
        for hh in range(4):
            h = hg * 4 + hh
            out[b, :, h * 64:(h + 1) * 64] = w[hh].T
    return out



# revision 11
# speedup vs baseline: 1.5059x; 1.5059x over previous
"""Trainium2 Bass kernel for causal cross-attention with L2-normalized q/k.

Reference computation (B=4, S=2048, E=512, H=8, Dh=64):
    q = k_embed @ Wq.T ; k = x @ Wk.T ; v = x @ Wv.T        (per batch)
    q,k l2-normalized over Dh per head; scores = g * q @ k.T (causal mask)
    out = softmax(scores) @ v

Sharding: 8 cores = 4 batches x 2 head-groups (4 heads each).

v2: all matmuls in bf16 (4x PE throughput vs fp32), inputs DMA'd as bf16
(half HBM traffic), attention processed in 512-wide sq blocks with both
heads of a pair packed into one PSUM tile so each Exp activation covers
two heads (fewer scalar-engine fixed costs; exp is the critical engine).
Projection tiles are interleaved with attention blocks so the scalar
engine starts exp work early. Output is written bf16 (host divides by
the softmax denominator row in f32).
"""

import numpy as np

B, S, E, H = 4, 2048, 512, 8
Dh = 64
NE = E // 128          # 4 contraction chunks
NT = S // 128          # 16 s-tiles
SQB = 512              # sq block width (1 PSUM bank per head)
NJ = S // SQB          # 4


def _build(g: float):
    from contextlib import ExitStack

    import concourse.tile as tile
    from concourse import bacc, mybir

    f32 = mybir.dt.float32
    bf16 = mybir.dt.bfloat16
    AF = mybir.ActivationFunctionType
    ALU = mybir.AluOpType

    nc = bacc.Bacc("TRN2", target_bir_lowering=False, debug=False)
    xT_d = nc.dram_tensor("xt", [E, S], bf16, kind="ExternalInput")
    keT_d = nc.dram_tensor("ket", [E, S], bf16, kind="ExternalInput")
    wqT_d = nc.dram_tensor("wqt", [E, 256], bf16, kind="ExternalInput")
    wvkT_d = nc.dram_tensor("wvkt", [E, 512], bf16, kind="ExternalInput")
    mask_d = nc.dram_tensor("mask01", [128, 2, 128], bf16, kind="ExternalInput")
    out_d = nc.dram_tensor("outt", [65, 4, S], bf16, kind="ExternalOutput")

    with tile.TileContext(nc) as tc, ExitStack() as ctx:
        persist = ctx.enter_context(tc.tile_pool(name="persist", bufs=1))
        ptmp = ctx.enter_context(tc.tile_pool(name="ptmp", bufs=2))
        ep = ctx.enter_context(tc.tile_pool(name="e_pool", bufs=3))
        osb = ctx.enter_context(tc.tile_pool(name="o_sb", bufs=2))
        pn = ctx.enter_context(tc.tile_pool(name="pn_ps", bufs=2, space="PSUM"))
        stp = ctx.enter_context(tc.tile_pool(name="st_ps", bufs=2, space="PSUM"))
        op = ctx.enter_context(tc.tile_pool(name="o_ps", bufs=1, space="PSUM"))

        # ---- input DMAs ----
        wq_sb = persist.tile([128, NE, 256], bf16, tag="wq")
        wvk_sb = persist.tile([128, NE, 512], bf16, tag="wvk")
        nc.sync.dma_start(out=wvk_sb, in_=wvkT_d.rearrange("(c p) m -> p c m", p=128))
        nc.sync.dma_start(out=wq_sb, in_=wqT_d.rearrange("(c p) m -> p c m", p=128))
        mask_sb = persist.tile([128, 2, 128], bf16, tag="mask")
        nc.sync.dma_start(out=mask_sb, in_=mask_d[:, :, :])

        x_sbs, ke_sbs = [], []
        for ec in range(NE):
            t = persist.tile([128, S], bf16, tag=f"x{ec}", name=f"x{ec}")
            nc.sync.dma_start(out=t[:, 0:1024], in_=xT_d[ec * 128:(ec + 1) * 128, 0:1024])
            nc.sync.dma_start(out=t[:, 1024:S], in_=xT_d[ec * 128:(ec + 1) * 128, 1024:S])
            x_sbs.append(t)
        for ec in range(NE):
            t = persist.tile([128, S], bf16, tag=f"ke{ec}", name=f"ke{ec}")
            nc.sync.dma_start(out=t[:, 0:1024], in_=keT_d[ec * 128:(ec + 1) * 128, 0:1024])
            nc.sync.dma_start(out=t[:, 1024:S], in_=keT_d[ec * 128:(ec + 1) * 128, 1024:S])
            ke_sbs.append(t)

        qt_sb = persist.tile([128, 2, S], bf16, tag="qt")   # qhat^T per pair
        kt_sb = persist.tile([128, 2, S], bf16, tag="kt")   # khat^T per pair
        v_sb = persist.tile([128, NT, 4, 65], bf16, tag="v")  # v + ones col
        nc.vector.memset(v_sb[:, :, :, 64], 1.0)

        def proj_tile(st_i):
            sl = slice(st_i * 128, (st_i + 1) * 128)
            psvk = pn.tile([128, 512], f32, tag="nat", name="nat")
            for ec in range(NE):
                nc.tensor.matmul(
                    psvk[:, :], lhsT=x_sbs[ec][:, sl], rhs=wvk_sb[:, ec, :],
                    start=(ec == 0), stop=(ec == NE - 1))
            nc.vector.tensor_copy(
                v_sb[:, st_i, :, 0:64],
                psvk[:, 0:256].rearrange("p (h d) -> p h d", h=4))
            psq = pn.tile([128, 512], f32, tag="nat", name="nat")
            for ec in range(NE):
                nc.tensor.matmul(
                    psq[:, 0:256], lhsT=ke_sbs[ec][:, sl], rhs=wq_sb[:, ec, :],
                    start=(ec == 0), stop=(ec == NE - 1))
            # raw q/k to SBUF (gpsimd cannot read PSUM; vector does the reads)
            qn = ptmp.tile([128, 4, 64], bf16, tag="qn", name="qn")
            nc.vector.tensor_copy(
                qn[:, :, :], psq[:, 0:256].rearrange("p (h d) -> p h d", h=4))
            kn = ptmp.tile([128, 4, 64], bf16, tag="kn", name="kn")
            nc.vector.tensor_copy(
                kn[:, :, :], psvk[:, 256:512].rearrange("p (h d) -> p h d", h=4))
            # sum-of-squares per (row, head) for q and k
            sqq = ptmp.tile([128, 4, 64], bf16, tag="sqq", name="sqq")
            nc.vector.tensor_mul(sqq[:, :, :], qn[:, :, :], qn[:, :, :])
            sqk = ptmp.tile([128, 4, 64], bf16, tag="sqk", name="sqk")
            nc.vector.tensor_mul(sqk[:, :, :], kn[:, :, :], kn[:, :, :])
            ss = ptmp.tile([128, 8], f32, tag="ss", name="ss")
            nc.vector.tensor_reduce(
                ss[:, 0:4], sqq[:, :, :], axis=mybir.AxisListType.X, op=ALU.add)
            nc.vector.tensor_reduce(
                ss[:, 4:8], sqk[:, :, :], axis=mybir.AxisListType.X, op=ALU.add)
            sn = ptmp.tile([128, 8], f32, tag="sn", name="sn")
            nc.scalar.activation(sn[:, :], ss[:, :], AF.Sqrt)
            rq = ptmp.tile([128, 8], f32, tag="rq", name="rq")
            nc.vector.reciprocal(rq[:, :], sn[:, :])
            # hat = raw * (1/norm), bf16 (gpsimd: all-SBUF)
            hatq = ptmp.tile([128, 4, 64], bf16, tag="hatq", name="hatq")
            hatk = ptmp.tile([128, 4, 64], bf16, tag="hatk", name="hatk")
            for h4 in range(4):
                nc.gpsimd.tensor_scalar_mul(
                    hatq[:, h4, :], qn[:, h4, :], rq[:, h4:h4 + 1])
            for h4 in range(4):
                nc.gpsimd.tensor_scalar_mul(
                    hatk[:, h4, :], kn[:, h4, :], rq[:, 4 + h4:5 + h4])
            for pair in range(2):
                nc.sync.dma_start_transpose(
                    qt_sb[:, pair, st_i * 128:(st_i + 1) * 128],
                    hatq[:, 2 * pair:2 * pair + 2, :])
                nc.sync.dma_start_transpose(
                    kt_sb[:, pair, st_i * 128:(st_i + 1) * 128],
                    hatk[:, 2 * pair:2 * pair + 2, :])

        def attn_block(pair, j, blk):
            ni = 4 * (j + 1)
            ops = op.tile([65, 2, SQB], f32, tag="o", name="o")
            for i in range(ni):
                co = max(0, 128 * i - SQB * j)
                corner = 128 * i >= SQB * j
                st = stp.tile([128, 2, SQB], f32, tag="st", name="st")
                for hp in range(2):
                    psl = slice(hp * 64, (hp + 1) * 64)
                    nc.tensor.matmul(
                        st[:, hp, co:SQB],
                        lhsT=kt_sb[psl, pair, i * 128:(i + 1) * 128],
                        rhs=qt_sb[psl, pair, j * SQB + co:(j + 1) * SQB],
                        start=True, stop=True)
                et = ep.tile([128, 2, SQB], bf16, tag="e", name="e")
                nc.scalar.activation(
                    et[:, :, co:SQB], st[:, :, co:SQB], AF.Exp, scale=float(g))
                if corner:
                    nc.vector.tensor_mul(
                        et[:, :, co:co + 128], et[:, :, co:co + 128],
                        mask_sb[:, :, :])
                for hp in range(2):
                    hh = 2 * pair + hp
                    nc.tensor.matmul(
                        ops[:, hp, co:SQB],
                        lhsT=v_sb[:, i, hh, :],
                        rhs=et[:, hp, co:SQB],
                        start=(i == 0), stop=(i == ni - 1))
            ot = osb.tile([65, 2, SQB], bf16, tag="osb", name="osb")
            nc.vector.tensor_copy(ot[:, :, :], ops[:, :, :])
            nc.sync.dma_start(
                out_d[:, 2 * pair:2 * pair + 2, j * SQB:(j + 1) * SQB],
                ot[:, :, :])

        # interleave projections with attention so exp starts early
        blk = 0
        for st_i in range(4):
            proj_tile(st_i)
        attn_block(0, 0, 0)
        for st_i in range(4, 8):
            proj_tile(st_i)
        attn_block(1, 0, 1)
        attn_block(0, 1, 2)
        for st_i in range(8, 12):
            proj_tile(st_i)
        attn_block(1, 1, 3)
        attn_block(0, 2, 4)
        for st_i in range(12, 16):
            proj_tile(st_i)
        attn_block(1, 2, 5)
        attn_block(0, 3, 6)
        attn_block(1, 3, 7)

    nc.compile()
    return nc


_NC_CACHE = {}


def _get_nc(g: float):
    if g not in _NC_CACHE:
        _NC_CACHE[g] = _build(g)
    return _NC_CACHE[g]


def _numpy_fallback(x, k_embed, attn_mask, key_padding_mask, Wq, Wk, Wv, g_scale):
    def l2n(t):
        n = np.sqrt((t * t).sum(-1, keepdims=True))
        return t / np.maximum(n, 1e-12)
    q = (k_embed @ Wq.T).reshape(B, S, H, Dh).transpose(0, 2, 1, 3)
    k = (x @ Wk.T).reshape(B, S, H, Dh).transpose(0, 2, 1, 3)
    v = (x @ Wv.T).reshape(B, S, H, Dh).transpose(0, 2, 1, 3)
    q, k = l2n(q), l2n(k)
    s = float(g_scale) * np.einsum('bhqd,bhkd->bhqk', q, k)
    s = np.where(attn_mask[None, None], -np.inf, s)
    s = np.where(key_padding_mask[:, None, None, :], -np.inf, s)
    s = s - s.max(-1, keepdims=True)
    e = np.exp(s)
    a = e / e.sum(-1, keepdims=True)
    o = np.einsum('bhqk,bhkd->bhqd', a, v)
    return o.transpose(0, 2, 1, 3).reshape(B, S, E).astype(np.float32)


def _make_in_maps(x, k_embed, Wq, Wk, Wv):
    import ml_dtypes
    bf = ml_dtypes.bfloat16
    m01 = np.triu(np.ones((128, 128), np.float32)).astype(bf)  # keep sq >= sk
    mask2 = np.ascontiguousarray(np.broadcast_to(m01[:, None, :], (128, 2, 128)))
    in_maps = []
    for c in range(8):
        b, hg = c // 2, c % 2
        rows = slice(hg * 256, (hg + 1) * 256)
        wv_t = Wv[rows].T                       # [512, 256]
        wk_t = Wk[rows].T
        in_maps.append({
            "xt": np.ascontiguousarray(x[b].T).astype(bf),
            "ket": np.ascontiguousarray(k_embed[b].T).astype(bf),
            "wqt": np.ascontiguousarray(Wq[rows].T).astype(bf),
            "wvkt": np.ascontiguousarray(
                np.concatenate([wv_t, wk_t], axis=1)).astype(bf),
            "mask01": mask2,
        })
    return in_maps


def kernel(**inputs) -> np.ndarray:
    x = np.asarray(inputs["x"], np.float32)
    k_embed = np.asarray(inputs["k_embed"], np.float32)
    attn_mask = np.asarray(inputs["attn_mask"])
    key_padding_mask = np.asarray(inputs["key_padding_mask"])
    Wq = np.asarray(inputs["Wq"], np.float32)
    Wk = np.asarray(inputs["Wk"], np.float32)
    Wv = np.asarray(inputs["Wv"], np.float32)
    g = float(np.asarray(inputs["g_scale"]))

    causal = np.triu(np.ones((S, S), bool), k=1)
    if (attn_mask != causal).any() or key_padding_mask.any():
        return _numpy_fallback(x, k_embed, attn_mask, key_padding_mask,
                               Wq, Wk, Wv, g)

    import os

    from concourse.bass_utils import run_bass_kernel_spmd

    nc = _get_nc(g)
    in_maps = _make_in_maps(x, k_embed, Wq, Wk, Wv)
    res = run_bass_kernel_spmd(nc, in_maps, core_ids=list(range(8)),
                               tmpdir=os.environ.get("BASS_NEFF_DIR"))
    kernel._last_results = res

    out = np.empty((B, S, E), np.float32)
    for c in range(8):
        b, hg = c // 2, c % 2
        r = res.results[c]["outt"].astype(np.float32)   # [65, 4, S]
        w = r[0:64, :, :] / r[64:65, :, :]              # normalize by denom
        for hh in range(4):
            h = hg * 4 + hh
            out[b, :, h * 64:(h + 1) * 64] = w[:, hh, :].T
    return out


# revision 13
# speedup vs baseline: 2.0866x; 1.3857x over previous
"""Trainium2 Bass kernel for causal cross-attention with L2-normalized q/k.

Reference computation (B=4, S=2048, E=512, H=8, Dh=64):
    q = k_embed @ Wq.T ; k = x @ Wk.T ; v = x @ Wv.T        (per batch)
    q,k l2-normalized over Dh per head; scores = g * q @ k.T (causal mask)
    out = softmax(scores) @ v

Sharding: 8 cores = 4 batches x 2 head-groups (4 heads each).

v2: all matmuls in bf16 (4x PE throughput vs fp32), inputs DMA'd as bf16
(half HBM traffic), attention processed in 512-wide sq blocks with both
heads of a pair packed into one PSUM tile so each Exp activation covers
two heads (fewer scalar-engine fixed costs; exp is the critical engine).
Projection tiles are interleaved with attention blocks so the scalar
engine starts exp work early. Output is written bf16 (host divides by
the softmax denominator row in f32).
"""

import numpy as np

B, S, E, H = 4, 2048, 512, 8
Dh = 64
NE = E // 128          # 4 contraction chunks
NT = S // 128          # 16 s-tiles
SQB = 512              # sq block width (1 PSUM bank per head)
NJ = S // SQB          # 4


def _build(g: float):
    from contextlib import ExitStack

    import concourse.tile as tile
    from concourse import bacc, mybir

    f32 = mybir.dt.float32
    bf16 = mybir.dt.bfloat16
    AF = mybir.ActivationFunctionType
    ALU = mybir.AluOpType

    nc = bacc.Bacc("TRN2", target_bir_lowering=False, debug=False)
    xT_d = nc.dram_tensor("xt", [E, S], bf16, kind="ExternalInput")
    keT_d = nc.dram_tensor("ket", [E, S], bf16, kind="ExternalInput")
    wqT_d = nc.dram_tensor("wqt", [E, 256], bf16, kind="ExternalInput")
    wvkT_d = nc.dram_tensor("wvkt", [E, 512], bf16, kind="ExternalInput")
    mask_d = nc.dram_tensor("mask01", [128, 2, 128], bf16, kind="ExternalInput")
    out_d = nc.dram_tensor("outt", [65, 4, S], bf16, kind="ExternalOutput")

    with tile.TileContext(nc) as tc, ExitStack() as ctx:
        persist = ctx.enter_context(tc.tile_pool(name="persist", bufs=1))
        ptmp = ctx.enter_context(tc.tile_pool(name="ptmp", bufs=2))
        ep = ctx.enter_context(tc.tile_pool(name="e_pool", bufs=3))
        osb = ctx.enter_context(tc.tile_pool(name="o_sb", bufs=2))
        pn = ctx.enter_context(tc.tile_pool(name="pn_ps", bufs=2, space="PSUM"))
        stp = ctx.enter_context(tc.tile_pool(name="st_ps", bufs=2, space="PSUM"))
        op = ctx.enter_context(tc.tile_pool(name="o_ps", bufs=1, space="PSUM"))

        # ---- input DMAs ----
        wq_sb = persist.tile([128, NE, 256], bf16, tag="wq")
        wvk_sb = persist.tile([128, NE, 512], bf16, tag="wvk")
        nc.sync.dma_start(out=wvk_sb, in_=wvkT_d.rearrange("(c p) m -> p c m", p=128))
        nc.sync.dma_start(out=wq_sb, in_=wqT_d.rearrange("(c p) m -> p c m", p=128))
        mask_sb = persist.tile([128, 2, 128], bf16, tag="mask")
        nc.sync.dma_start(out=mask_sb, in_=mask_d[:, :, :])

        x_sbs, ke_sbs = [], []
        for ec in range(NE):
            t = persist.tile([128, S], bf16, tag=f"x{ec}", name=f"x{ec}")
            nc.sync.dma_start(out=t[:, 0:1024], in_=xT_d[ec * 128:(ec + 1) * 128, 0:1024])
            nc.sync.dma_start(out=t[:, 1024:S], in_=xT_d[ec * 128:(ec + 1) * 128, 1024:S])
            x_sbs.append(t)
        for ec in range(NE):
            t = persist.tile([128, S], bf16, tag=f"ke{ec}", name=f"ke{ec}")
            nc.sync.dma_start(out=t[:, 0:1024], in_=keT_d[ec * 128:(ec + 1) * 128, 0:1024])
            nc.sync.dma_start(out=t[:, 1024:S], in_=keT_d[ec * 128:(ec + 1) * 128, 1024:S])
            ke_sbs.append(t)

        qt_sb = persist.tile([128, 2, S], bf16, tag="qt")   # qhat^T per pair
        kt_sb = persist.tile([128, 2, S], bf16, tag="kt")   # khat^T per pair
        v_sb = persist.tile([128, NT, 4, 65], bf16, tag="v")  # v + ones col
        nc.vector.memset(v_sb[:, :, :, 64], 1.0)

        def proj_tile(st_i):
            sl = slice(st_i * 128, (st_i + 1) * 128)
            psvk = pn.tile([128, 512], f32, tag="nat", name="nat")
            for ec in range(NE):
                nc.tensor.matmul(
                    psvk[:, :], lhsT=x_sbs[ec][:, sl], rhs=wvk_sb[:, ec, :],
                    start=(ec == 0), stop=(ec == NE - 1))
            nc.vector.tensor_copy(
                v_sb[:, st_i, :, 0:64],
                psvk[:, 0:256].rearrange("p (h d) -> p h d", h=4))
            psq = pn.tile([128, 512], f32, tag="nat", name="nat")
            for ec in range(NE):
                nc.tensor.matmul(
                    psq[:, 0:256], lhsT=ke_sbs[ec][:, sl], rhs=wq_sb[:, ec, :],
                    start=(ec == 0), stop=(ec == NE - 1))
            # raw q/k to SBUF (gpsimd cannot read PSUM; vector does the reads)
            qn = ptmp.tile([128, 4, 64], bf16, tag="qn", name="qn")
            nc.vector.tensor_copy(
                qn[:, :, :], psq[:, 0:256].rearrange("p (h d) -> p h d", h=4))
            kn = ptmp.tile([128, 4, 64], bf16, tag="kn", name="kn")
            nc.vector.tensor_copy(
                kn[:, :, :], psvk[:, 256:512].rearrange("p (h d) -> p h d", h=4))
            # sum-of-squares per (row, head) for q and k
            sqq = ptmp.tile([128, 4, 64], bf16, tag="sqq", name="sqq")
            nc.vector.tensor_mul(sqq[:, :, :], qn[:, :, :], qn[:, :, :])
            sqk = ptmp.tile([128, 4, 64], bf16, tag="sqk", name="sqk")
            nc.vector.tensor_mul(sqk[:, :, :], kn[:, :, :], kn[:, :, :])
            ss = ptmp.tile([128, 8], f32, tag="ss", name="ss")
            nc.vector.tensor_reduce(
                ss[:, 0:4], sqq[:, :, :], axis=mybir.AxisListType.X, op=ALU.add)
            nc.vector.tensor_reduce(
                ss[:, 4:8], sqk[:, :, :], axis=mybir.AxisListType.X, op=ALU.add)
            # rsqrt = exp(-0.5*ln(x)): Ln and Exp live in the same ACT table
            # set (natural_log_exp_and_others), so no table reloads between
            # these and the attention exps.
            lns = ptmp.tile([128, 8], f32, tag="lns", name="lns")
            nc.scalar.activation(lns[:, :], ss[:, :], AF.Ln)
            rq = ptmp.tile([128, 8], f32, tag="rq", name="rq")
            nc.scalar.activation(rq[:, :], lns[:, :], AF.Exp, scale=-0.5)
            # hat = raw * (1/norm), bf16 — one broadcast multiply per tensor
            hatq = ptmp.tile([128, 4, 64], bf16, tag="hatq", name="hatq")
            hatk = ptmp.tile([128, 4, 64], bf16, tag="hatk", name="hatk")
            nc.vector.tensor_mul(
                hatq[:, :, :], qn[:, :, :],
                rq[:, 0:4].broadcast_to([128, 4, 64]))
            nc.vector.tensor_mul(
                hatk[:, :, :], kn[:, :, :],
                rq[:, 4:8].broadcast_to([128, 4, 64]))
            for pair in range(2):
                nc.sync.dma_start_transpose(
                    qt_sb[:, pair, st_i * 128:(st_i + 1) * 128],
                    hatq[:, 2 * pair:2 * pair + 2, :])
                nc.sync.dma_start_transpose(
                    kt_sb[:, pair, st_i * 128:(st_i + 1) * 128],
                    hatk[:, 2 * pair:2 * pair + 2, :])

        def attn_block(pair, j, blk):
            ni = 4 * (j + 1)
            ops = op.tile([65, 2, SQB], f32, tag="o", name="o")

            def scores(i):
                co = max(0, 128 * i - SQB * j)
                st = stp.tile([128, 2, SQB], f32, tag="st", name="st")
                for hp in range(2):
                    psl = slice(hp * 64, (hp + 1) * 64)
                    nc.tensor.matmul(
                        st[:, hp, co:SQB],
                        lhsT=kt_sb[psl, pair, i * 128:(i + 1) * 128],
                        rhs=qt_sb[psl, pair, j * SQB + co:(j + 1) * SQB],
                        start=True, stop=True)
                return st

            def exp_av(i, st):
                co = max(0, 128 * i - SQB * j)
                corner = 128 * i >= SQB * j
                et = ep.tile([128, 2, SQB], bf16, tag="e", name="e")
                nc.scalar.activation(
                    et[:, :, co:SQB], st[:, :, co:SQB], AF.Exp, scale=float(g))
                if corner:
                    nc.vector.tensor_mul(
                        et[:, :, co:co + 128], et[:, :, co:co + 128],
                        mask_sb[:, :, :])
                for hp in range(2):
                    hh = 2 * pair + hp
                    nc.tensor.matmul(
                        ops[:, hp, co:SQB],
                        lhsT=v_sb[:, i, hh, :],
                        rhs=et[:, hp, co:SQB],
                        start=(i == 0), stop=(i == ni - 1))

            # software pipeline: scores(i+1) issues on the PE queue before
            # AV(i), so the PE streams scores while the scalar engine exps
            prev = None
            for i in range(ni):
                st = scores(i)
                if prev is not None:
                    exp_av(prev[0], prev[1])
                prev = (i, st)
            exp_av(prev[0], prev[1])
            ot = osb.tile([65, 2, SQB], bf16, tag="osb", name="osb")
            nc.vector.tensor_copy(ot[:, :, :], ops[:, :, :])
            nc.sync.dma_start(
                out_d[:, 2 * pair:2 * pair + 2, j * SQB:(j + 1) * SQB],
                ot[:, :, :])

        # interleave projections with attention so exp starts early
        blk = 0
        for st_i in range(4):
            proj_tile(st_i)
        attn_block(0, 0, 0)
        for st_i in range(4, 8):
            proj_tile(st_i)
        attn_block(1, 0, 1)
        attn_block(0, 1, 2)
        for st_i in range(8, 12):
            proj_tile(st_i)
        attn_block(1, 1, 3)
        attn_block(0, 2, 4)
        for st_i in range(12, 16):
            proj_tile(st_i)
        attn_block(1, 2, 5)
        attn_block(0, 3, 6)
        attn_block(1, 3, 7)

    nc.compile()
    return nc


_NC_CACHE = {}


def _get_nc(g: float):
    if g not in _NC_CACHE:
        _NC_CACHE[g] = _build(g)
    return _NC_CACHE[g]


def _numpy_fallback(x, k_embed, attn_mask, key_padding_mask, Wq, Wk, Wv, g_scale):
    def l2n(t):
        n = np.sqrt((t * t).sum(-1, keepdims=True))
        return t / np.maximum(n, 1e-12)
    q = (k_embed @ Wq.T).reshape(B, S, H, Dh).transpose(0, 2, 1, 3)
    k = (x @ Wk.T).reshape(B, S, H, Dh).transpose(0, 2, 1, 3)
    v = (x @ Wv.T).reshape(B, S, H, Dh).transpose(0, 2, 1, 3)
    q, k = l2n(q), l2n(k)
    s = float(g_scale) * np.einsum('bhqd,bhkd->bhqk', q, k)
    s = np.where(attn_mask[None, None], -np.inf, s)
    s = np.where(key_padding_mask[:, None, None, :], -np.inf, s)
    s = s - s.max(-1, keepdims=True)
    e = np.exp(s)
    a = e / e.sum(-1, keepdims=True)
    o = np.einsum('bhqk,bhkd->bhqd', a, v)
    return o.transpose(0, 2, 1, 3).reshape(B, S, E).astype(np.float32)


def _make_in_maps(x, k_embed, Wq, Wk, Wv):
    import ml_dtypes
    bf = ml_dtypes.bfloat16
    m01 = np.triu(np.ones((128, 128), np.float32)).astype(bf)  # keep sq >= sk
    mask2 = np.ascontiguousarray(np.broadcast_to(m01[:, None, :], (128, 2, 128)))
    in_maps = []
    for c in range(8):
        b, hg = c // 2, c % 2
        rows = slice(hg * 256, (hg + 1) * 256)
        wv_t = Wv[rows].T                       # [512, 256]
        wk_t = Wk[rows].T
        in_maps.append({
            "xt": np.ascontiguousarray(x[b].T).astype(bf),
            "ket": np.ascontiguousarray(k_embed[b].T).astype(bf),
            "wqt": np.ascontiguousarray(Wq[rows].T).astype(bf),
            "wvkt": np.ascontiguousarray(
                np.concatenate([wv_t, wk_t], axis=1)).astype(bf),
            "mask01": mask2,
        })
    return in_maps


def kernel(**inputs) -> np.ndarray:
    x = np.asarray(inputs["x"], np.float32)
    k_embed = np.asarray(inputs["k_embed"], np.float32)
    attn_mask = np.asarray(inputs["attn_mask"])
    key_padding_mask = np.asarray(inputs["key_padding_mask"])
    Wq = np.asarray(inputs["Wq"], np.float32)
    Wk = np.asarray(inputs["Wk"], np.float32)
    Wv = np.asarray(inputs["Wv"], np.float32)
    g = float(np.asarray(inputs["g_scale"]))

    causal = np.triu(np.ones((S, S), bool), k=1)
    if (attn_mask != causal).any() or key_padding_mask.any():
        return _numpy_fallback(x, k_embed, attn_mask, key_padding_mask,
                               Wq, Wk, Wv, g)

    import os

    from concourse.bass_utils import run_bass_kernel_spmd

    nc = _get_nc(g)
    in_maps = _make_in_maps(x, k_embed, Wq, Wk, Wv)
    res = run_bass_kernel_spmd(nc, in_maps, core_ids=list(range(8)),
                               tmpdir=os.environ.get("BASS_NEFF_DIR"))
    kernel._last_results = res

    out = np.empty((B, S, E), np.float32)
    for c in range(8):
        b, hg = c // 2, c % 2
        r = res.results[c]["outt"].astype(np.float32)   # [65, 4, S]
        w = r[0:64, :, :] / r[64:65, :, :]              # normalize by denom
        for hh in range(4):
            h = hg * 4 + hh
            out[b, :, h * 64:(h + 1) * 64] = w[:, hh, :].T
    return out


# revision 14
# speedup vs baseline: 2.2402x; 1.0736x over previous
"""Trainium2 Bass kernel for causal cross-attention with L2-normalized q/k.

Reference computation (B=4, S=2048, E=512, H=8, Dh=64):
    q = k_embed @ Wq.T ; k = x @ Wk.T ; v = x @ Wv.T        (per batch)
    q,k l2-normalized over Dh per head; scores = g * q @ k.T (causal mask)
    out = softmax(scores) @ v

Sharding: 8 cores = 4 batches x 2 head-groups (4 heads each).

v2: all matmuls in bf16 (4x PE throughput vs fp32), inputs DMA'd as bf16
(half HBM traffic), attention processed in 512-wide sq blocks with both
heads of a pair packed into one PSUM tile so each Exp activation covers
two heads (fewer scalar-engine fixed costs; exp is the critical engine).
Projection tiles are interleaved with attention blocks so the scalar
engine starts exp work early. Output is written bf16 (host divides by
the softmax denominator row in f32).
"""

import numpy as np

B, S, E, H = 4, 2048, 512, 8
Dh = 64
NE = E // 128          # 4 contraction chunks
NT = S // 128          # 16 s-tiles
SQB = 512              # sq block width (1 PSUM bank per head)
NJ = S // SQB          # 4


def _build(g: float):
    from contextlib import ExitStack

    import concourse.tile as tile
    from concourse import bacc, mybir

    f32 = mybir.dt.float32
    bf16 = mybir.dt.bfloat16
    AF = mybir.ActivationFunctionType
    ALU = mybir.AluOpType

    nc = bacc.Bacc("TRN2", target_bir_lowering=False, debug=False)
    xT_d = nc.dram_tensor("xt", [E, S], bf16, kind="ExternalInput")
    keT_d = nc.dram_tensor("ket", [E, S], bf16, kind="ExternalInput")
    wqT_d = nc.dram_tensor("wqt", [E, 256], bf16, kind="ExternalInput")
    wvkT_d = nc.dram_tensor("wvkt", [E, 512], bf16, kind="ExternalInput")
    mask_d = nc.dram_tensor("mask01", [128, 2, 128], bf16, kind="ExternalInput")
    out_d = nc.dram_tensor("outt", [65, 4, S], bf16, kind="ExternalOutput")

    with tile.TileContext(nc) as tc, ExitStack() as ctx:
        persist = ctx.enter_context(tc.tile_pool(name="persist", bufs=1))
        ptmp = ctx.enter_context(tc.tile_pool(name="ptmp", bufs=2))
        ep = ctx.enter_context(tc.tile_pool(name="e_pool", bufs=3))
        osb = ctx.enter_context(tc.tile_pool(name="o_sb", bufs=2))
        pn = ctx.enter_context(tc.tile_pool(name="pn_ps", bufs=2, space="PSUM"))
        stp = ctx.enter_context(tc.tile_pool(name="st_ps", bufs=2, space="PSUM"))
        op = ctx.enter_context(tc.tile_pool(name="o_ps", bufs=1, space="PSUM"))

        # ---- input DMAs ----
        wq_sb = persist.tile([128, NE, 256], bf16, tag="wq")
        wvk_sb = persist.tile([128, NE, 512], bf16, tag="wvk")
        nc.sync.dma_start(out=wvk_sb, in_=wvkT_d.rearrange("(c p) m -> p c m", p=128))
        nc.sync.dma_start(out=wq_sb, in_=wqT_d.rearrange("(c p) m -> p c m", p=128))
        mask_sb = persist.tile([128, 2, 128], bf16, tag="mask")
        nc.sync.dma_start(out=mask_sb, in_=mask_d[:, :, :])

        x_sbs, ke_sbs = [], []
        for ec in range(NE):
            t = persist.tile([128, S], bf16, tag=f"x{ec}", name=f"x{ec}")
            nc.sync.dma_start(out=t[:, 0:1024], in_=xT_d[ec * 128:(ec + 1) * 128, 0:1024])
            nc.sync.dma_start(out=t[:, 1024:S], in_=xT_d[ec * 128:(ec + 1) * 128, 1024:S])
            x_sbs.append(t)
        for ec in range(NE):
            t = persist.tile([128, S], bf16, tag=f"ke{ec}", name=f"ke{ec}")
            nc.sync.dma_start(out=t[:, 0:1024], in_=keT_d[ec * 128:(ec + 1) * 128, 0:1024])
            nc.sync.dma_start(out=t[:, 1024:S], in_=keT_d[ec * 128:(ec + 1) * 128, 1024:S])
            ke_sbs.append(t)

        qt_sb = persist.tile([128, 2, S], bf16, tag="qt")   # qhat^T per pair
        kt_sb = persist.tile([128, 2, S], bf16, tag="kt")   # khat^T per pair
        v_sb = persist.tile([128, NT, 4, 65], bf16, tag="v")  # v + ones col
        nc.vector.memset(v_sb[:, :, :, 64], 1.0)

        def proj_tile(st_i):
            sl = slice(st_i * 128, (st_i + 1) * 128)
            psvk = pn.tile([128, 512], f32, tag="nat", name="nat")
            for ec in range(NE):
                nc.tensor.matmul(
                    psvk[:, :], lhsT=x_sbs[ec][:, sl], rhs=wvk_sb[:, ec, :],
                    start=(ec == 0), stop=(ec == NE - 1))
            nc.vector.tensor_copy(
                v_sb[:, st_i, :, 0:64],
                psvk[:, 0:256].rearrange("p (h d) -> p h d", h=4))
            psq = pn.tile([128, 512], f32, tag="nat", name="nat")
            for ec in range(NE):
                nc.tensor.matmul(
                    psq[:, 0:256], lhsT=ke_sbs[ec][:, sl], rhs=wq_sb[:, ec, :],
                    start=(ec == 0), stop=(ec == NE - 1))
            # raw q/k to SBUF (gpsimd cannot read PSUM; vector does the reads)
            qn = ptmp.tile([128, 4, 64], bf16, tag="qn", name="qn")
            nc.vector.tensor_copy(
                qn[:, :, :], psq[:, 0:256].rearrange("p (h d) -> p h d", h=4))
            kn = ptmp.tile([128, 4, 64], bf16, tag="kn", name="kn")
            nc.vector.tensor_copy(
                kn[:, :, :], psvk[:, 256:512].rearrange("p (h d) -> p h d", h=4))
            # sum-of-squares per (row, head) for q and k
            sqq = ptmp.tile([128, 4, 64], bf16, tag="sqq", name="sqq")
            nc.vector.tensor_mul(sqq[:, :, :], qn[:, :, :], qn[:, :, :])
            sqk = ptmp.tile([128, 4, 64], bf16, tag="sqk", name="sqk")
            nc.vector.tensor_mul(sqk[:, :, :], kn[:, :, :], kn[:, :, :])
            ss = ptmp.tile([128, 8], f32, tag="ss", name="ss")
            nc.vector.tensor_reduce(
                ss[:, 0:4], sqq[:, :, :], axis=mybir.AxisListType.X, op=ALU.add)
            nc.vector.tensor_reduce(
                ss[:, 4:8], sqk[:, :, :], axis=mybir.AxisListType.X, op=ALU.add)
            # rsqrt = exp(-0.5*ln(x)): Ln and Exp live in the same ACT table
            # set (natural_log_exp_and_others), so no table reloads between
            # these and the attention exps.
            lns = ptmp.tile([128, 8], f32, tag="lns", name="lns")
            nc.scalar.activation(lns[:, :], ss[:, :], AF.Ln)
            rq = ptmp.tile([128, 8], f32, tag="rq", name="rq")
            nc.scalar.activation(rq[:, :], lns[:, :], AF.Exp, scale=-0.5)
            # hat = raw * (1/norm), bf16 — one broadcast multiply per tensor
            hatq = ptmp.tile([128, 4, 64], bf16, tag="hatq", name="hatq")
            hatk = ptmp.tile([128, 4, 64], bf16, tag="hatk", name="hatk")
            nc.vector.tensor_mul(
                hatq[:, :, :], qn[:, :, :],
                rq[:, 0:4].broadcast_to([128, 4, 64]))
            nc.vector.tensor_mul(
                hatk[:, :, :], kn[:, :, :],
                rq[:, 4:8].broadcast_to([128, 4, 64]))
            for pair in range(2):
                nc.sync.dma_start_transpose(
                    qt_sb[:, pair, st_i * 128:(st_i + 1) * 128],
                    hatq[:, 2 * pair:2 * pair + 2, :])
                nc.sync.dma_start_transpose(
                    kt_sb[:, pair, st_i * 128:(st_i + 1) * 128],
                    hatk[:, 2 * pair:2 * pair + 2, :])

        def attn_block(pair, j, blk):
            ni = 4 * (j + 1)
            ops = op.tile([65, 2, SQB], f32, tag="o", name="o")

            def scores(i):
                co = max(0, 128 * i - SQB * j)
                st = stp.tile([128, 2, SQB], f32, tag="st", name="st")
                for hp in range(2):
                    psl = slice(hp * 64, (hp + 1) * 64)
                    nc.tensor.matmul(
                        st[:, hp, co:SQB],
                        lhsT=kt_sb[psl, pair, i * 128:(i + 1) * 128],
                        rhs=qt_sb[psl, pair, j * SQB + co:(j + 1) * SQB],
                        start=True, stop=True)
                return st

            def exp_av(i, st):
                co = max(0, 128 * i - SQB * j)
                corner = 128 * i >= SQB * j
                et = ep.tile([128, 2, SQB], bf16, tag="e", name="e")
                nc.scalar.activation(
                    et[:, :, co:SQB], st[:, :, co:SQB], AF.Exp, scale=float(g))
                if corner:
                    nc.vector.tensor_mul(
                        et[:, :, co:co + 128], et[:, :, co:co + 128],
                        mask_sb[:, :, :])
                for hp in range(2):
                    hh = 2 * pair + hp
                    nc.tensor.matmul(
                        ops[:, hp, co:SQB],
                        lhsT=v_sb[:, i, hh, :],
                        rhs=et[:, hp, co:SQB],
                        start=(i == 0), stop=(i == ni - 1))

            # software pipeline: scores(i+1) issues on the PE queue before
            # AV(i), so the PE streams scores while the scalar engine exps
            prev = None
            for i in range(ni):
                st = scores(i)
                if prev is not None:
                    exp_av(prev[0], prev[1])
                prev = (i, st)
            exp_av(prev[0], prev[1])
            ot = osb.tile([65, 2, SQB], bf16, tag="osb", name="osb")
            nc.vector.tensor_copy(ot[:, :, :], ops[:, :, :])
            nc.sync.dma_start(
                out_d[:, 2 * pair:2 * pair + 2, j * SQB:(j + 1) * SQB],
                ot[:, :, :])

        # interleave projections with attention so exp starts early
        blk = 0
        for st_i in range(4):
            proj_tile(st_i)
        attn_block(0, 0, 0)
        for st_i in range(4, 8):
            proj_tile(st_i)
        attn_block(1, 0, 1)
        attn_block(0, 1, 2)
        for st_i in range(8, 12):
            proj_tile(st_i)
        attn_block(1, 1, 3)
        attn_block(0, 2, 4)
        for st_i in range(12, 16):
            proj_tile(st_i)
        attn_block(1, 2, 5)
        attn_block(0, 3, 6)
        attn_block(1, 3, 7)

    nc.compile()

    # The act-table pass maps Exp -> exp_and_others and Ln -> natural_log,
    # alternating ~30 table reloads (~1.3us each) on the scalar engine.
    # natural_log_exp_and_others contains both functions, so retarget every
    # load to it and keep only the first load per block (loads carry no
    # semaphore wiring; walrus adopts pre-placed loads for inlined BIR).
    from concourse.hw_specs import get_activation_tables
    tabs = list(get_activation_tables(nc.m.arch))
    target = tabs.index("natural_log_exp_and_others")
    for blk in nc.m.functions[0].blocks:
        seen = False
        drop = []
        for idx, inst in enumerate(blk.instructions):
            if isinstance(inst, mybir.InstLoadActFuncSet):
                assert inst.sync_info is None or (
                    not inst.sync_info.on_wait and not inst.sync_info.on_update)
                if seen:
                    drop.append(idx)
                else:
                    inst.act_func_set_id = target
                    seen = True
        for idx in reversed(drop):
            del blk.instructions[idx]
    return nc


_NC_CACHE = {}


def _get_nc(g: float):
    if g not in _NC_CACHE:
        _NC_CACHE[g] = _build(g)
    return _NC_CACHE[g]


def _numpy_fallback(x, k_embed, attn_mask, key_padding_mask, Wq, Wk, Wv, g_scale):
    def l2n(t):
        n = np.sqrt((t * t).sum(-1, keepdims=True))
        return t / np.maximum(n, 1e-12)
    q = (k_embed @ Wq.T).reshape(B, S, H, Dh).transpose(0, 2, 1, 3)
    k = (x @ Wk.T).reshape(B, S, H, Dh).transpose(0, 2, 1, 3)
    v = (x @ Wv.T).reshape(B, S, H, Dh).transpose(0, 2, 1, 3)
    q, k = l2n(q), l2n(k)
    s = float(g_scale) * np.einsum('bhqd,bhkd->bhqk', q, k)
    s = np.where(attn_mask[None, None], -np.inf, s)
    s = np.where(key_padding_mask[:, None, None, :], -np.inf, s)
    s = s - s.max(-1, keepdims=True)
    e = np.exp(s)
    a = e / e.sum(-1, keepdims=True)
    o = np.einsum('bhqk,bhkd->bhqd', a, v)
    return o.transpose(0, 2, 1, 3).reshape(B, S, E).astype(np.float32)


def _make_in_maps(x, k_embed, Wq, Wk, Wv):
    import ml_dtypes
    bf = ml_dtypes.bfloat16
    m01 = np.triu(np.ones((128, 128), np.float32)).astype(bf)  # keep sq >= sk
    mask2 = np.ascontiguousarray(np.broadcast_to(m01[:, None, :], (128, 2, 128)))
    in_maps = []
    for c in range(8):
        b, hg = c // 2, c % 2
        rows = slice(hg * 256, (hg + 1) * 256)
        wv_t = Wv[rows].T                       # [512, 256]
        wk_t = Wk[rows].T
        in_maps.append({
            "xt": np.ascontiguousarray(x[b].T).astype(bf),
            "ket": np.ascontiguousarray(k_embed[b].T).astype(bf),
            "wqt": np.ascontiguousarray(Wq[rows].T).astype(bf),
            "wvkt": np.ascontiguousarray(
                np.concatenate([wv_t, wk_t], axis=1)).astype(bf),
            "mask01": mask2,
        })
    return in_maps


def kernel(**inputs) -> np.ndarray:
    x = np.asarray(inputs["x"], np.float32)
    k_embed = np.asarray(inputs["k_embed"], np.float32)
    attn_mask = np.asarray(inputs["attn_mask"])
    key_padding_mask = np.asarray(inputs["key_padding_mask"])
    Wq = np.asarray(inputs["Wq"], np.float32)
    Wk = np.asarray(inputs["Wk"], np.float32)
    Wv = np.asarray(inputs["Wv"], np.float32)
    g = float(np.asarray(inputs["g_scale"]))

    causal = np.triu(np.ones((S, S), bool), k=1)
    if (attn_mask != causal).any() or key_padding_mask.any():
        return _numpy_fallback(x, k_embed, attn_mask, key_padding_mask,
                               Wq, Wk, Wv, g)

    import os

    from concourse.bass_utils import run_bass_kernel_spmd

    nc = _get_nc(g)
    in_maps = _make_in_maps(x, k_embed, Wq, Wk, Wv)
    res = run_bass_kernel_spmd(nc, in_maps, core_ids=list(range(8)),
                               tmpdir=os.environ.get("BASS_NEFF_DIR"))
    kernel._last_results = res

    out = np.empty((B, S, E), np.float32)
    for c in range(8):
        b, hg = c // 2, c % 2
        r = res.results[c]["outt"].astype(np.float32)   # [65, 4, S]
        w = r[0:64, :, :] / r[64:65, :, :]              # normalize by denom
        for hh in range(4):
            h = hg * 4 + hh
            out[b, :, h * 64:(h + 1) * 64] = w[:, hh, :].T
    return out
